# revision 37
# baseline (speedup 1.0000x reference)
"""Trainium2 Bass kernel for nn_DisplacementTensors (gnn_message_passing).

Math (per edge e with displacement r and source node n = src[e]):
    d   = |r|
    rbf = exp(-((d - c_k)/w)^2)            k=0..7
    h   = rbf @ w_rad + b_rad
    rad = h @ w_direct + (lrelu(lrelu(h@w1+b1)@w2+b2) @ w3 + b3)   # [32]
    s   = 7 / sqrt(1 + 49 d^2)             (tens_sigmoid scale)
    t   = [1, s*rx, s*ry, s*rz, s2*xx, s2*xy, s2*xz, s2*yy, s2*yz, s2*zz]
    A[n] += rad (outer) t                  # [32, 10] per node
    A_a = A[:, :, 0]
    out_v = w_v^T A[:, :, 1:4]
    out_d = w_d^T A[:, :, 4:10] (symmetric 3x3 expand)

Distribution: nodes are dealt round-robin by degree rank to 8 cores (so all
cores share one static schedule); edges of a node all go to its core. Per
core, edges are sorted by node position and packed into 128-edge tiles that
never span a 51-node PSUM window; each tile does one matmul
  psum[32, 10*G] += rad_tile[128e, 32]^T @ t_exp[128e, 10*G]
accumulating at column offset 10*base into the window's PSUM bank, where
t_exp[e, g*10+j] = t[e, j] * (slot[e] == g).  Per-edge MLP runs feature-major
on TensorE (2 edge-groups packed on partitions); rbf/rad are transposed
between edge-major and feature-major layouts by bouncing through DRAM.
"""

import numpy as np

# ---------------- problem constants (hardcoded per contract) ----------------
N_NODES = 20000
N_EDGES = 640000
DIM_A = 32
R0 = 1.0
NCORES = 8
NPC = N_NODES // NCORES          # 2500 nodes per core

# ---------------- scheme knobs ----------------
WN = 51                          # nodes per psum window (51*10=510 <= 512)
G = 6                            # node slots per tile
TILE = 128                       # edges per tile
SC = 1024                        # feature-major sub-chunk (2 groups x 512)
EM_CHUNK_T = 128                 # edge-major chunk, in tiles

_cache = {}


# ============================================================================
# Host-side schedule construction
# ============================================================================
def _build_schedule(edges_src):
    """Returns dict with the common (core-independent) tile schedule and the
    per-core edge placement."""
    deg = np.bincount(edges_src, minlength=N_NODES)
    rank = np.argsort(-deg, kind="stable")       # node ids, descending degree
    idx = np.arange(N_NODES)
    p = idx // NCORES
    j = idx % NCORES
    core = np.where(p % 2 == 0, j, NCORES - 1 - j)
    core_of = np.empty(N_NODES, np.int64)
    pos_of = np.empty(N_NODES, np.int64)
    core_of[rank] = core
    pos_of[rank] = p
    node_at = np.empty((NCORES, NPC), np.int64)
    node_at[core, p] = rank

    edge_order = np.argsort(edges_src, kind="stable")
    node_start = np.zeros(N_NODES + 1, np.int64)
    node_start[1:] = np.cumsum(deg)

    n_windows = (NPC + WN - 1) // WN
    tiles_w = []                                 # (window, base)
    core_eidx = [[] for _ in range(NCORES)]      # per tile: [128] global edge ids / -1
    core_slot = [[] for _ in range(NCORES)]      # per tile: [128] slot 0..G-1 / -1

    pad_e = -np.ones(TILE, np.int64)

    for w in range(n_windows):
        w0 = w * WN
        wlen = min(WN, NPC - w0)
        nodes_w = node_at[:, w0:w0 + wlen]                      # [8, wlen]
        degs_w = deg[nodes_w]                                   # [8, wlen]
        cum_w = np.cumsum(degs_w, axis=1)                       # ends per node
        total_w = cum_w[:, -1]
        # per-core flat edge list for this window, node-position order
        ew = []
        for k in range(NCORES):
            segs = [edge_order[node_start[n]:node_start[n + 1]]
                    for n in nodes_w[k]]
            ew.append(np.concatenate(segs) if segs else np.empty(0, np.int64))
        ptr = np.zeros(NCORES, np.int64)
        covered = np.zeros(wlen, bool)
        base_cap = max(0, wlen - G)
        while np.any(ptr < total_w):
            # frontier node-rel for each unfinished core
            fr = [int(np.searchsorted(cum_w[k], ptr[k], side="right"))
                  for k in range(NCORES) if ptr[k] < total_w[k]]
            base = min(min(fr), base_cap)
            hi_node = min(base + G, wlen)                       # exclusive
            for k in range(NCORES):
                hi_edge = cum_w[k, hi_node - 1]
                take = int(min(TILE, max(0, hi_edge - ptr[k])))
                if take > 0:
                    ids = ew[k][ptr[k]:ptr[k] + take]
                    rel = np.searchsorted(cum_w[k],
                                          np.arange(ptr[k], ptr[k] + take),
                                          side="right")
                    slot = rel - base
                    e_arr = pad_e.copy(); e_arr[:take] = ids
                    s_arr = pad_e.copy(); s_arr[:take] = slot
                    ptr[k] += take
                else:
                    e_arr = pad_e
                    s_arr = pad_e
                core_eidx[k].append(e_arr)
                core_slot[k].append(s_arr)
            covered[base:base + G] = True
            tiles_w.append((w, base))
        # coverage tiles for node-positions untouched by any tile
        rel = 0
        while rel < wlen:
            if covered[rel]:
                rel += 1
                continue
            b = min(rel, base_cap)
            tiles_w.append((w, b))
            for k in range(NCORES):
                core_eidx[k].append(pad_e)
                core_slot[k].append(pad_e)
            covered[b:b + G] = True
            rel = b + G

    # pad tile count to a multiple of SC//TILE (so fm sub-chunks are full)
    mult = SC // TILE
    while len(tiles_w) % mult:
        tiles_w.append((n_windows - 1, 0))
        for k in range(NCORES):
            core_eidx[k].append(pad_e)
            core_slot[k].append(pad_e)

    T = len(tiles_w)
    win_id = np.array([t[0] for t in tiles_w], np.int64)
    base = np.array([t[1] for t in tiles_w], np.int64)
    # first/last tile index per window (tiles of one window are contiguous
    # except for the final pad tiles which also map to the last window; treat
    # first occurrence / last occurrence)
    is_first = np.zeros(T, bool)
    is_last = np.zeros(T, bool)
    seen = set()
    for i in range(T):
        if win_id[i] not in seen:
            seen.add(win_id[i])
            is_first[i] = True
    seen = set()
    for i in range(T - 1, -1, -1):
        if win_id[i] not in seen:
            seen.add(win_id[i])
            is_last[i] = True

    eidx = [np.concatenate(core_eidx[k]) for k in range(NCORES)]   # [T*128]
    slot = [np.concatenate(core_slot[k]) for k in range(NCORES)]
    return dict(deg=deg, node_at=node_at, n_windows=n_windows, T=T,
                win_id=win_id, base=base, is_first=is_first, is_last=is_last,
                eidx=eidx, slot=slot)


def _build_planes(sched, r_ij):
    """Per-core fp32 planes: x, y, z, slot."""
    planes = []
    for k in range(NCORES):
        e = sched["eidx"][k]
        s = sched["slot"][k]
        valid = e >= 0
        xyz = np.zeros((3, e.size), np.float32)
        xyz[:, valid] = r_ij[e[valid]].T
        sl = s.astype(np.float32)          # -1 for pads
        planes.append(dict(x=xyz[0], y=xyz[1], z=xyz[2], slot=sl))
    return planes


# ============================================================================
# Numpy simulation of the exact device scheme (for validation in test.py)
# ============================================================================
def _numpy_scheme(sched, planes, weights):
    """Runs the same tiled math in numpy; returns (A_a, out_v, out_d) full."""
    w_rad, b_rad = weights["w_rad"], weights["b_rad"]
    w_direct, w1, b1 = weights["w_direct"], weights["w1"], weights["b1"]
    w2, b2, w3, b3 = weights["w2"], weights["b2"], weights["w3"], weights["b3"]
    w_v, w_d = weights["w_v"], weights["w_d"]
    lrelu = lambda x: np.where(x > 0, x, 0.1 * x)

    A_a = np.zeros((N_NODES, DIM_A), np.float32)
    out_v = np.zeros((N_NODES, DIM_A, 3), np.float32)
    out_d = np.zeros((N_NODES, DIM_A, 3, 3), np.float32)
    sym = np.array([[0, 1, 2], [1, 3, 4], [2, 4, 5]])

    T = sched["T"]
    for k in range(NCORES):
        pl = planes[k]
        x, y, z, sl = pl["x"], pl["y"], pl["z"], pl["slot"]
        d2 = x * x + y * y + z * z
        d = np.sqrt(d2)
        centers = np.linspace(0.0, R0, 8).astype(np.float32)
        wdt = R0 / 8.0
        rbf = np.exp(-(((d[:, None] - centers) / wdt) ** 2)).astype(np.float32)
        h = rbf @ w_rad + b_rad
        rad = h @ w_direct + (lrelu(lrelu(h @ w1 + b1) @ w2 + b2) @ w3 + b3)
        inv = 1.0 / (1.0 + 49.0 * d2)
        s = 7.0 * np.sqrt(inv)
        s2 = 49.0 * inv
        t = np.stack([np.ones_like(x), s * x, s * y, s * z,
                      s2 * x * x, s2 * x * y, s2 * x * z,
                      s2 * y * y, s2 * y * z, s2 * z * z], axis=1)  # [L,10]
        # accumulate windows
        Awin = {}
        for i in range(T):
            w = sched["win_id"][i]
            b = sched["base"][i]
            if w not in Awin:
                Awin[w] = np.zeros((DIM_A, 512), np.float32)
            sl_t = sl[i * TILE:(i + 1) * TILE]
            t_t = t[i * TILE:(i + 1) * TILE]
            rad_t = rad[i * TILE:(i + 1) * TILE]
            texp = np.zeros((TILE, 10 * G), np.float32)
            for g in range(G):
                m = (sl_t == g).astype(np.float32)
                texp[:, g * 10:(g + 1) * 10] = t_t * m[:, None]
            Awin[w][:, 10 * b:10 * b + 10 * G] += rad_t.T @ texp
        for w, A in Awin.items():
            w0 = w * WN
            wlen = min(WN, NPC - w0)
            nodes = sched["node_at"][k, w0:w0 + wlen]
            blk = A[:, :10 * wlen].reshape(DIM_A, wlen, 10)
            A_a[nodes] = blk[:, :, 0].T
            Av = blk[:, :, 1:4]                      # [32, wlen, 3]
            Ad = blk[:, :, 4:10]                     # [32, wlen, 6]
            ov = np.einsum("av,anc->nvc", w_v, Av)
            od6 = np.einsum("ad,ank->ndk", w_d, Ad)  # [wlen,32,6]
            out_v[nodes] = ov
            out_d[nodes] = od6[:, :, sym]
    return A_a, out_v, out_d


# ============================================================================
# Device program
# ============================================================================
def _stack2(w, off):
    """[in,out] -> [off+in, out] with a second copy at partition `off`, so the
    group-1 weights start at the same partition as the group-1 activations."""
    fin, fout = w.shape
    out = np.zeros((off + fin, fout), np.float32)
    out[:fin] = w
    out[off:off + fin] = w
    return out


def _build_program(sched, dtype_mm="bf16"):
    from contextlib import ExitStack
    import concourse.bass as bass
    import concourse.bacc as bacc
    import concourse.tile as tile
    from concourse import mybir

    dt = mybir.dt
    mm_dt = dt.bfloat16 if dtype_mm == "bf16" else dt.float32
    f32 = dt.float32

    T = sched["T"]
    L = T * TILE                      # padded edges per core
    n_windows = sched["n_windows"]
    win_id, base = sched["win_id"], sched["base"]
    is_first, is_last = sched["is_first"], sched["is_last"]
    NSC = L // SC                     # feature-major sub-chunks
    NEC = (T + EM_CHUNK_T - 1) // EM_CHUNK_T

    nc = bacc.Bacc("TRN2", target_bir_lowering=False, debug=False)

    # ---- dram parameters ----
    def din(name, shape):
        return nc.declare_dram_parameter(name, list(shape), f32, isOutput=False)

    def dinw(name, shape):
        return nc.declare_dram_parameter(name, list(shape), mm_dt,
                                         isOutput=False)

    x_d = din("x", (L,)); y_d = din("y", (L,)); z_d = din("z", (L,))
    sl_d = din("slot", (L,))
    # all weights packed into two tensors (keeps preamble to 2 DMAs):
    # bf16 [128, 288]: w_rad@0 w_dir@32 w1@64 w2@128 w3@192 w_v@224 w_d@256
    # f32  [128, 13]:  b_rad@0 b1@1 b2@2 b3@3 rbf_bias@4..11 lrelu_alpha@12
    wpack_d = dinw("wpack", (128, 288))
    bpack_d = din("bpack", (128, 13))

    oa_d = nc.declare_dram_parameter("out_a", [32, NPC], f32, isOutput=True)
    ov_d = nc.declare_dram_parameter("out_v", [32, NPC * 3], f32, isOutput=True)
    od_d = nc.declare_dram_parameter("out_d", [32, NPC * 6], f32, isOutput=True)

    rbf_dram = nc.dram_tensor("rbf_scratch", [8, L], mm_dt)
    rad_dram = nc.dram_tensor("rad_scratch", [32, L], mm_dt)

    centers = np.linspace(0.0, R0, 8)
    wdt = R0 / 8.0

    with tile.TileContext(nc) as tc, ExitStack() as ctx:
        wpool = ctx.enter_context(tc.tile_pool(name="weights", bufs=1))
        empool = ctx.enter_context(tc.tile_pool(name="em", bufs=2))
        fmpool = ctx.enter_context(tc.tile_pool(name="fm", bufs=2))
        pspool = ctx.enter_context(tc.tile_pool(name="ps", bufs=1, space="PSUM"))
        winpool = ctx.enter_context(tc.tile_pool(name="win", bufs=2, space="PSUM"))
        prjpool = ctx.enter_context(tc.tile_pool(name="prj", bufs=1, space="PSUM"))
        accpool = ctx.enter_context(tc.tile_pool(name="acc", bufs=1))
        opool = ctx.enter_context(tc.tile_pool(name="out", bufs=3))

        # ---- load weights (2 packed DMAs) ----
        wsb = wpool.tile([128, 288], mm_dt, tag="wsb")
        bsb = wpool.tile([128, 13], f32, tag="bsb")
        nc.sync.dma_start(out=wsb[:], in_=wpack_d[:])
        nc.sync.dma_start(out=bsb[:], in_=bpack_d[:])
        w_rad = wsb[:, 0:32]
        w_dir = wsb[:, 32:64]
        w1 = wsb[:, 64:128]
        w2 = wsb[:, 128:192]
        w3 = wsb[:, 192:224]
        w_v = wsb[0:32, 224:256]
        w_d = wsb[0:32, 256:288]
        b_rad = bsb[:, 0:1]
        b1 = bsb[:, 1:2]
        b2 = bsb[:, 2:3]
        b3 = bsb[:, 3:4]
        rbf_bias = bsb[:, 4:12]
        lr_alpha = bsb[:, 12:13]

        # full-core A accumulator (feature-major, [32, NPC*10]) in bf16 for
        # projections; A_a is written per-window in fp32 directly.
        A_full = accpool.tile([32, NPC * 10], mm_dt, tag="A_full")
        # whole-core expanded t (consumed by phase 3)
        texp_all = accpool.tile([128, T, G, 10], mm_dt, tag="texp_all")

        # ================= phase 1: edge-major elementwise =================
        # per chunk of EM_CHUNK_T tiles: load planes, compute rbf (-> dram)
        # and t/t_exp (t_exp kept whole-core in SBUF for phase 3).
        for c in range(NEC):
            t0 = c * EM_CHUNK_T
            tn = min(EM_CHUNK_T, T - t0)
            n = tn * TILE
            sl_ = slice(t0 * TILE, t0 * TILE + n)

            xs = empool.tile([128, tn], f32, tag="xs")
            ys = empool.tile([128, tn], f32, tag="ys")
            zs = empool.tile([128, tn], f32, tag="zs")
            ss = empool.tile([128, tn], f32, tag="ss")
            nc.sync.dma_start(out=xs[:], in_=x_d[sl_].rearrange("(t p) -> p t", p=128))
            nc.sync.dma_start(out=ys[:], in_=y_d[sl_].rearrange("(t p) -> p t", p=128))
            nc.sync.dma_start(out=zs[:], in_=z_d[sl_].rearrange("(t p) -> p t", p=128))
            nc.sync.dma_start(out=ss[:], in_=sl_d[sl_].rearrange("(t p) -> p t", p=128))

            d2 = empool.tile([128, tn], f32, tag="d2")
            tmp = empool.tile([128, tn], f32, tag="tmp")
            nc.vector.tensor_mul(d2[:], xs[:], xs[:])
            nc.vector.tensor_mul(tmp[:], ys[:], ys[:])
            nc.vector.tensor_add(d2[:], d2[:], tmp[:])
            nc.vector.tensor_mul(tmp[:], zs[:], zs[:])
            nc.vector.tensor_add(d2[:], d2[:], tmp[:])
            dd = empool.tile([128, tn], f32, tag="dd")
            nc.scalar.sqrt(dd[:], d2[:])

            # rbf_k = exp(-((d-c_k)/w)^2) : square((d - c)/w) then exp(-u)
            arg = empool.tile([128, tn, 8], f32, tag="arg")
            for k8 in range(8):
                nc.scalar.activation(
                    arg[:, :, k8], dd[:],
                    mybir.ActivationFunctionType.Square,
                    bias=rbf_bias[:, k8:k8 + 1], scale=float(1.0 / wdt))
            rbf = empool.tile([128, tn, 8], mm_dt, tag="rbf")
            nc.scalar.activation(
                rbf[:], arg[:], mybir.ActivationFunctionType.Exp,
                bias=0.0, scale=-1.0)
            for k8 in range(8):
                nc.sync.dma_start(
                    out=rbf_dram[k8, sl_].rearrange("(t p) -> p t", p=128),
                    in_=rbf[:, :, k8])

            # s = 7*sqrt(inv), s2 = 49*inv, inv = 1/(1+49 d2)
            inv = empool.tile([128, tn], f32, tag="inv")
            nc.vector.tensor_scalar(tmp[:], d2[:], 49.0, 1.0,
                                    mybir.AluOpType.mult, mybir.AluOpType.add)
            nc.vector.reciprocal(inv[:], tmp[:])
            s_t = empool.tile([128, tn], f32, tag="s_t")
            nc.scalar.activation(s_t[:], inv[:],
                                 mybir.ActivationFunctionType.Sqrt,
                                 bias=0.0, scale=49.0)

            # t[p, t, 10]: [1, s*r, s2*rr]
            tt = empool.tile([128, tn, 10], f32, tag="tt")
            nc.vector.memset(tt[:, :, 0], 1.0)
            nc.vector.tensor_mul(tt[:, :, 1], s_t[:], xs[:])
            nc.vector.tensor_mul(tt[:, :, 2], s_t[:], ys[:])
            nc.vector.tensor_mul(tt[:, :, 3], s_t[:], zs[:])
            # rr entries: use already-scaled t1..t3: s2*x*y = (s*x)*(s*y)*inv... no:
            # (s*x)*(s*y) = s^2*x*y = s2*x*y exactly. reuse t columns.
            nc.vector.tensor_mul(tt[:, :, 4], tt[:, :, 1], tt[:, :, 1])
            nc.vector.tensor_mul(tt[:, :, 5], tt[:, :, 1], tt[:, :, 2])
            nc.vector.tensor_mul(tt[:, :, 6], tt[:, :, 1], tt[:, :, 3])
            nc.vector.tensor_mul(tt[:, :, 7], tt[:, :, 2], tt[:, :, 2])
            nc.vector.tensor_mul(tt[:, :, 8], tt[:, :, 2], tt[:, :, 3])
            nc.vector.tensor_mul(tt[:, :, 9], tt[:, :, 3], tt[:, :, 3])

            # mask[p, t, g] = (slot == g), bf16
            mask = empool.tile([128, tn, G], mm_dt, tag="mask")
            for g in range(G):
                nc.vector.tensor_scalar(mask[:, :, g], ss[:], float(g), None,
                                        mybir.AluOpType.is_equal)
            # t_exp[p, t, g, 10] = t * mask_g
            for g in range(G):
                nc.vector.tensor_tensor(
                    out=texp_all[:, t0:t0 + tn, g, :],
                    in0=tt[:],
                    in1=mask[:, :, g].unsqueeze(2).broadcast_to([128, tn, 10]),
                    op=mybir.AluOpType.mult)

        # ================= phase 2: feature-major MLP =================
        Lr = mybir.ActivationFunctionType.Prelu
        Idt = mybir.ActivationFunctionType.Identity
        for sc in range(NSC):
            e0 = sc * SC
            g0 = slice(e0, e0 + 512)
            g1 = slice(e0 + 512, e0 + SC)
            rbf_fm = fmpool.tile([40, 512], mm_dt, tag="rbf_fm")
            nc.sync.dma_start(out=rbf_fm[0:8, :], in_=rbf_dram[:, g0])
            nc.sync.dma_start(out=rbf_fm[32:40, :], in_=rbf_dram[:, g1])

            hps = pspool.tile([64, 512], f32, tag="hps")
            nc.tensor.matmul(hps[0:32, :], w_rad[0:8, :], rbf_fm[0:8, :],
                             start=True, stop=True)
            nc.tensor.matmul(hps[32:64, :], w_rad[32:40, :], rbf_fm[32:40, :],
                             start=True, stop=True, tile_position=(32, 32))
            h_sb = fmpool.tile([64, 512], mm_dt, tag="h_sb")
            nc.scalar.activation(h_sb[:], hps[:], Idt, bias=b_rad[0:64, :], scale=1.0)

            rps = pspool.tile([64, 512], f32, tag="rps")
            nc.tensor.matmul(rps[0:32, :], w_dir[0:32, :], h_sb[0:32, :],
                             start=True, stop=False)
            nc.tensor.matmul(rps[32:64, :], w_dir[32:64, :], h_sb[32:64, :],
                             start=True, stop=False, tile_position=(32, 32))

            a1ps = pspool.tile([128, 512], f32, tag="a1ps")
            nc.tensor.matmul(a1ps[0:64, :], w1[0:32, :], h_sb[0:32, :],
                             start=True, stop=True)
            nc.tensor.matmul(a1ps[64:128, :], w1[32:64, :], h_sb[32:64, :],
                             start=True, stop=True, tile_position=(32, 64))
            a1_sb = fmpool.tile([128, 512], mm_dt, tag="a1_sb")
            nc.scalar.activation(a1_sb[:], a1ps[:], Lr, bias=b1[:, :], scale=1.0,
                                 alpha=lr_alpha)

            a2ps = pspool.tile([128, 512], f32, tag="a2ps")
            nc.tensor.matmul(a2ps[0:64, :], w2[0:64, :], a1_sb[0:64, :],
                             start=True, stop=True)
            nc.tensor.matmul(a2ps[64:128, :], w2[64:128, :], a1_sb[64:128, :],
                             start=True, stop=True, tile_position=(64, 64))
            a2_sb = fmpool.tile([128, 512], mm_dt, tag="a2_sb")
            nc.scalar.activation(a2_sb[:], a2ps[:], Lr, bias=b2[:, :], scale=1.0,
                                 alpha=lr_alpha)

            nc.tensor.matmul(rps[0:32, :], w3[0:64, :], a2_sb[0:64, :],
                             start=False, stop=True)
            nc.tensor.matmul(rps[32:64, :], w3[64:128, :], a2_sb[64:128, :],
                             start=False, stop=True, tile_position=(64, 32))
            rad_sb = fmpool.tile([64, 512], mm_dt, tag="rad_sb")
            nc.scalar.activation(rad_sb[:], rps[:], Idt, bias=b3[0:64, :], scale=1.0)
            nc.sync.dma_start(out=rad_dram[:, g0], in_=rad_sb[0:32, :])
            nc.sync.dma_start(out=rad_dram[:, g1], in_=rad_sb[32:64, :])

        # ================= phase 3: segment matmuls =================
        cur_win_ps = None
        for c in range(NEC):
            t0 = c * EM_CHUNK_T
            tn = min(EM_CHUNK_T, T - t0)
            sl_ = slice(t0 * TILE, t0 * TILE + tn * TILE)
            rad_em = empool.tile([128, tn, 32], mm_dt, tag="rad_em")
            for a8 in range(32):
                nc.sync.dma_start(
                    out=rad_em[:, :, a8],
                    in_=rad_dram[a8, sl_].rearrange("(t p) -> p t", p=128))
            for i in range(t0, t0 + tn):
                ti = i - t0
                w = int(win_id[i]); b = int(base[i])
                if is_first[i]:
                    cur_win_ps = winpool.tile([32, 512], f32, tag="winps")
                nc.tensor.matmul(
                    cur_win_ps[:, 10 * b:10 * b + 10 * G],
                    rad_em[:, ti, :], texp_all[:, i, :, :],
                    start=bool(is_first[i]), stop=bool(is_last[i]),
                    skip_group_check=True)
                if is_last[i]:
                    w0 = w * WN
                    wlen = min(WN, NPC - w0)
                    # full window -> A_full (bf16) for projections
                    nc.vector.tensor_copy(
                        out=A_full[:, 10 * w0:10 * (w0 + wlen)],
                        in_=cur_win_ps[:, :10 * wlen])
                    # A_a (j=0 columns) -> fp32 staging -> DRAM
                    a_sb = opool.tile([32, WN], f32, tag="a_sb")
                    nc.scalar.activation(
                        a_sb[:, :wlen],
                        cur_win_ps[:, :10 * wlen].rearrange("p (n j) -> p n j", j=10)[:, :, 0],
                        Idt, bias=0.0, scale=1.0)
                    nc.sync.dma_start(out=oa_d[:, w0:w0 + wlen],
                                      in_=a_sb[:, :wlen])

        # ================= phase 4: projections =================
        AF = A_full[:].rearrange("p (n j) -> p n j", j=10)
        NODE_CH = 64                      # nodes per projection chunk
        for n0 in range(0, NPC, NODE_CH):
            nn = min(NODE_CH, NPC - n0)
            pv = prjpool.tile([32, NODE_CH, 3], f32, tag="pv")
            nc.tensor.matmul(pv[:, :nn, :], w_v,
                             AF[:, n0:n0 + nn, 1:4],
                             start=True, stop=True)
            v_sb = opool.tile([32, NODE_CH, 3], f32, tag="v_sb")
            nc.scalar.copy(v_sb[:, :nn, :], pv[:, :nn, :])
            nc.sync.dma_start(out=ov_d[:, 3 * n0:3 * (n0 + nn)],
                              in_=v_sb[:, :nn, :])
            pd = prjpool.tile([32, NODE_CH, 6], f32, tag="pd")
            nc.tensor.matmul(pd[:, :nn, :], w_d,
                             AF[:, n0:n0 + nn, 4:10],
                             start=True, stop=True)
            d_sb = opool.tile([32, NODE_CH, 6], f32, tag="d_sb")
            nc.scalar.copy(d_sb[:, :nn, :], pd[:, :nn, :])
            nc.sync.dma_start(out=od_d[:, 6 * n0:6 * (n0 + nn)],
                              in_=d_sb[:, :nn, :])

    nc.compile()
    return nc


# ============================================================================
# Entry point
# ============================================================================
def _weights_dict(inputs):
    return {k: np.asarray(inputs[k], np.float32) for k in
            ["w_rad", "b_rad", "w_direct", "w1", "b1", "w2", "b2",
             "w3", "b3", "w_v", "w_d"]}


def _in_maps(sched, planes, weights):
    import ml_dtypes
    bf16 = ml_dtypes.bfloat16

    def pad128(a):
        out = np.zeros((128, a.shape[1]), np.float32)
        out[:a.shape[0]] = a
        return out

    wr = np.zeros((128, 32), np.float32)   # rows 0-7 g0, 32-39 g1
    wr[0:8] = weights["w_rad"]
    wr[32:40] = weights["w_rad"]
    wpack = np.concatenate([
        wr,
        pad128(_stack2(weights["w_direct"], 32)),
        pad128(_stack2(weights["w1"], 32)),
        pad128(np.concatenate([weights["w2"]] * 2)),
        pad128(np.concatenate([weights["w3"]] * 2)),
        pad128(weights["w_v"]),
        pad128(weights["w_d"]),
    ], axis=1).astype(bf16)
    assert wpack.shape == (128, 288)
    centers = np.linspace(0.0, R0, 8)
    wdt = R0 / 8.0
    bpack = np.zeros((128, 13), np.float32)
    bpack[0:64, 0] = np.tile(weights["b_rad"], 2)
    bpack[:, 1] = np.tile(weights["b1"], 2)
    bpack[:, 2] = np.tile(weights["b2"], 2)
    bpack[0:64, 3] = np.tile(weights["b3"], 2)
    bpack[:, 4:12] = (-centers / wdt)[None, :]
    bpack[:, 12] = 0.1
    wm = {"wpack": wpack, "bpack": bpack}
    maps = []
    for k in range(NCORES):
        m = dict(wm)
        m["x"] = planes[k]["x"]
        m["y"] = planes[k]["y"]
        m["z"] = planes[k]["z"]
        m["slot"] = planes[k]["slot"]
        maps.append(m)
    return maps


def _assemble(sched, results):
    sym = np.array([[0, 1, 2], [1, 3, 4], [2, 4, 5]])
    A_a = np.zeros((N_NODES, DIM_A), np.float32)
    out_v = np.zeros((N_NODES, DIM_A, 3), np.float32)
    out_d = np.zeros((N_NODES, DIM_A, 3, 3), np.float32)
    for k in range(NCORES):
        nodes = sched["node_at"][k]
        A_a[nodes] = results[k]["out_a"].T
        v = results[k]["out_v"].reshape(DIM_A, NPC, 3)
        out_v[nodes] = v.transpose(1, 0, 2)
        d6 = results[k]["out_d"].reshape(DIM_A, NPC, 6).transpose(1, 0, 2)
        out_d[nodes] = d6[:, :, sym]
    return A_a, out_v, out_d


def _run_pjrt(nc, in_maps, repeats=1):
    """Like bass2jax.run_bass_via_pjrt (multi-core branch), but jits once and
    can re-execute to measure steady-state device wall time."""
    import time
    import jax
    import numpy as _np
    from jax.sharding import Mesh, PartitionSpec
    from jax.experimental.shard_map import shard_map
    from concourse import bass2jax, mybir

    bass2jax.install_neuronx_cc_hook()
    n_cores = len(in_maps)
    partition_name = (nc.partition_id_tensor.name
                      if nc.partition_id_tensor else None)
    in_names, out_names, out_avals, zero_outs = [], [], [], []
    for alloc in nc.m.functions[0].allocations:
        if not isinstance(alloc, mybir.MemoryLocationSet):
            continue
        name = alloc.memorylocations[0].name
        if alloc.kind == "ExternalInput":
            if name != partition_name:
                in_names.append(name)
        elif alloc.kind == "ExternalOutput":
            out_names.append(name)
            shape = tuple(alloc.tensor_shape)
            dtype = mybir.dt.np(alloc.dtype)
            out_avals.append(jax.core.ShapedArray(shape, dtype))
            zero_outs.append(_np.zeros(shape, dtype))
    n_params = len(in_names)
    n_outs = len(out_avals)
    in_names_all = in_names + out_names
    if partition_name is not None:
        in_names_all.append(partition_name)

    def _body(*args):
        operands = list(args)
        if partition_name is not None:
            operands.append(bass2jax.partition_id_tensor())
        outs = bass2jax._bass_exec_p.bind(
            *operands, out_avals=tuple(out_avals),
            in_names=tuple(in_names_all), out_names=tuple(out_names),
            lowering_input_output_aliases=(), sim_require_finite=True,
            sim_require_nnan=True, nc=nc)
        return tuple(outs)

    devices = jax.devices()[:n_cores]
    mesh = Mesh(_np.asarray(devices), ("core",))
    in_specs = (PartitionSpec("core"),) * (n_params + n_outs)
    out_specs = (PartitionSpec("core"),) * n_outs
    sharded = jax.jit(
        shard_map(_body, mesh=mesh, in_specs=in_specs, out_specs=out_specs,
                  check_rep=False),
        keep_unused=True)
    concat_in = [
        _np.concatenate([_np.asarray(in_maps[c][nm]) for c in range(n_cores)],
                        axis=0)
        for nm in in_names]
    concat_zeros = [_np.zeros((n_cores * z.shape[0], *z.shape[1:]), z.dtype)
                    for z in zero_outs]
    dev_in = [jax.device_put(a) for a in concat_in + concat_zeros]
    out_arrs = None
    for rep in range(repeats):
        t0 = time.time()
        out_arrs = sharded(*dev_in)
        jax.block_until_ready(out_arrs)
        dt_ns = (time.time() - t0) * 1e9
        print(f"exec wall[{rep}]: {dt_ns:.0f} ns", flush=True)
        if rep == repeats - 1 and repeats > 1:
            print(f"HW exec time: {dt_ns:.0f} ns", flush=True)
    return [
        {nm: _np.asarray(out_arrs[i]).reshape(n_cores, *out_avals[i].shape)[c]
         for i, nm in enumerate(out_names)}
        for c in range(n_cores)]


def kernel(r_ij, w_rad, b_rad, w_direct, w1, b1, w2, b2, w3, b3, w_v, w_d,
           edges_src, n_nodes, _repeats=1):
    r_ij = np.asarray(r_ij, np.float32)
    edges_src = np.asarray(edges_src).astype(np.int64)
    weights = _weights_dict(dict(w_rad=w_rad, b_rad=b_rad, w_direct=w_direct,
                                 w1=w1, b1=b1, w2=w2, b2=b2, w3=w3, b3=b3,
                                 w_v=w_v, w_d=w_d))

    key = hash(edges_src.tobytes())
    if key not in _cache:
        sched = _build_schedule(edges_src)
        prog = _build_program(sched)
        _cache[key] = (sched, prog)
    sched, prog = _cache[key]
    planes = _build_planes(sched, r_ij)
    maps = _in_maps(sched, planes, weights)

    results = _run_pjrt(prog, maps, repeats=_repeats)
    return _assemble(sched, results)
